# revision 1
# baseline (speedup 1.0000x reference)
"""Trainium2 Bass kernel for the combined loss (KL + CE + InfoNCE + focal + adv CE).

v2 strategy (8 NeuronCores, data-parallel over the batch):
  - InfoNCE exploits Gram symmetry.  The 8192x8192 cosine-similarity matrix is
    a 64x64 grid of 128x128 blocks.  Every block-row r computes only the blocks
    at circulant distance d = 0..32 (columns (r+d) mod 64).  Each computed
    exp-block feeds TWO row sums: its own rows via the ScalarEngine's fused
    exp + row-accumulate, and the mirrored rows (distance 64-d) via a
    ones-matmul column-sum accumulated in PSUM by the PE.  This halves both
    the exp work (the baseline bottleneck) and the Gram matmuls.
  - Features are normalized on the host and shipped as bf16 [256, 8704]
    (columns rolled per core by 512*c and extended by 512 so the circulant
    sweep is contiguous; one SPMD program serves all cores).
  - CE/KL/focal/adv: per-sample stats ([128,1] accumulators) are computed
    on-device; the tiny per-row nonlinear epilogue (log/focal buckets) runs
    on the host on the gathered 4096-row stats.
  - Each core ships ~70KB of partial sums; the host reduces and applies the
    loss weights.
"""

import numpy as np
import ml_dtypes

import concourse.bacc as bacc
import concourse.tile as tile
from concourse import mybir
from concourse.bass_utils import run_bass_kernel_spmd

F32 = mybir.dt.float32
BF16 = mybir.dt.bfloat16
AF = mybir.ActivationFunctionType
ALU = mybir.AluOpType
AX = mybir.AxisListType

NCORES = 8
B, C, D = 4096, 1000, 256
RB = B // NCORES          # 512 rows of the [B, C] tensors per core
NT = RB // 128            # 4 row-tiles per core
N2 = 2 * B                # 8192 infoNCE rows
NDIST = 33                # circulant distances d = 0..32 per block-row
SPAN = NDIST * 128        # 4224 columns per block-row sweep
L_ROWS = [0, 1, 2, 3, 32, 33, 34, 35]   # local block-row indices (all cores)
EXT = 35 * 128 + SPAN     # 8704 extended local columns
CHW = 1536                # gram/exp chunk width (3 PSUM banks)
NCHK = (EXT + CHW - 1) // CHW           # 6 chunks
RGW = 512                 # colsum accumulation region width (1 PSUM bank)
NREG = EXT // RGW         # 17 regions

KL_TEMP = 4.0
KL_INTERP = 0.5
NCE_TEMP = 0.07
NEG_BIG = -1.0e9


def _pair_table():
    """(l, c, a, b) for every (block-row, chunk) intersection, chunk-major."""
    pairs = []
    for c in range(NCHK):
        c0, c1 = CHW * c, min(CHW * (c + 1), EXT)
        for l in L_ROWS:
            s, e = 128 * l, 128 * l + SPAN
            a, b = max(c0, s), min(c1, e)
            if b > a:
                pairs.append((l, c, a, b))
    return pairs


PAIRS = _pair_table()
NSLOT = len(PAIRS)        # 28


def _region_pieces():
    """region -> list of (pair_idx, p0, p1) colsum pieces, in emission order."""
    reg = {}
    for idx, (l, c, a, b) in enumerate(PAIRS):
        a2 = max(a, 128 * l + 128)        # exclude d=0 (diagonal block)
        b2 = min(b, 128 * l + 4096)       # exclude d=32 (rowsum-only block)
        if b2 <= a2:
            continue
        r0, r1 = a2 // RGW, (b2 - 1) // RGW
        for r in range(r0, r1 + 1):
            p0, p1 = max(a2, RGW * r), min(b2, RGW * (r + 1))
            reg.setdefault(r, []).append((idx, p0, p1))
    return reg


REGION_PIECES = _region_pieces()


def _build_module():
    nc = bacc.Bacc("TRN2", target_bir_lowering=False, debug=False)

    # packed per-tile CE/KL input: o(f32) | m(bf16) | a(f32) | tg | ta bytes
    OMA_W = 4 * C + 2 * C + 4 * C + 32
    oma_d = nc.dram_tensor("oma", [RB, OMA_W], mybir.dt.uint8,
                           kind="ExternalInput")
    ft_d = nc.dram_tensor("ft", [256, EXT], BF16, kind="ExternalInput")
    out_d = nc.dram_tensor("out", [128, 61], F32, kind="ExternalOutput")
    cs_d = nc.dram_tensor("cs", [1, EXT], F32, kind="ExternalOutput")

    iota_np = np.tile(np.arange(C, dtype=np.float32), (128, 1))
    # packed constants: ident f32 | identb bf16 | negidb | onesb | zerob
    cpack = np.concatenate([
        np.eye(128, dtype=np.float32).view(np.uint8).reshape(128, -1),
        np.eye(128).astype(ml_dtypes.bfloat16).view(np.uint8).reshape(128, -1),
        (NEG_BIG * np.eye(128)).astype(ml_dtypes.bfloat16).view(np.uint8).reshape(128, -1),
        np.ones((128, 128)).astype(ml_dtypes.bfloat16).view(np.uint8).reshape(128, -1),
        np.zeros((128, RGW)).astype(ml_dtypes.bfloat16).view(np.uint8).reshape(128, -1),
    ], axis=1)
    iota_d = nc.inline_tensor(iota_np, "iota_c")
    cpack_d = nc.inline_tensor(cpack, "cpack_c")

    with tile.TileContext(nc) as tc:
        with (
            tc.tile_pool(name="persist", bufs=1) as persist,
            tc.tile_pool(name="io", bufs=1) as iop,
            tc.tile_pool(name="scr", bufs=2) as scrp,
            tc.tile_pool(name="et", bufs=16) as etp,
            tc.tile_pool(name="vec", bufs=1) as vecp,
            tc.tile_pool(name="gp", bufs=2, space="PSUM") as gpp,
            tc.tile_pool(name="cs", bufs=2, space="PSUM") as csp,
        ):
            dma = nc.sync.dma_start

            cpack_t = persist.tile([128, cpack.shape[1]], mybir.dt.uint8,
                                   tag="cpack")
            ident_t = cpack_t[:, 0:512].bitcast(F32)
            identb_t = cpack_t[:, 512:768].bitcast(BF16)
            negidb_t = cpack_t[:, 768:1024].bitcast(BF16)
            onesb_t = cpack_t[:, 1024:1280].bitcast(BF16)
            zerob_t = cpack_t[:, 1280:1280 + 2 * RGW].bitcast(BF16)

            h0b = persist.tile([128, EXT], BF16, tag="h0b")
            h1b = persist.tile([128, EXT], BF16, tag="h1b")
            iota_t = persist.tile([128, C], F32, tag="iota")
            o_ts, m_ts, a_ts, oma_ts = [], [], [], []
            for t in range(NT):
                oma_t = iop.tile([128, OMA_W], mybir.dt.uint8, tag=f"oma{t}")
                oma_ts.append(oma_t)
                o_ts.append(oma_t[:, 0:4 * C].bitcast(F32))
                m_ts.append(oma_t[:, 4 * C:6 * C].bitcast(BF16))
                a_ts.append(oma_t[:, 6 * C:10 * C].bitcast(F32))
            tg_t = oma_ts[0][:, 10 * C:10 * C + 16].bitcast(F32)
            ta_t = oma_ts[0][:, 10 * C + 16:10 * C + 32].bitcast(F32)

            # progressive feature pieces so the first gram starts early
            P0, P1, P2 = 512, 1536, 4608
            dma(out=h0b[:, 0:P0], in_=ft_d[0:128, 0:P0])
            dma(out=h1b[:, 0:P0], in_=ft_d[128:256, 0:P0])
            dma(out=cpack_t[:], in_=cpack_d[:])
            dma(out=h0b[:, P0:P1], in_=ft_d[0:128, P0:P1])
            dma(out=h1b[:, P0:P1], in_=ft_d[128:256, P0:P1])
            dma(out=h0b[:, P1:P2], in_=ft_d[0:128, P1:P2])
            dma(out=h1b[:, P1:P2], in_=ft_d[128:256, P1:P2])
            dma(out=iota_t[:], in_=iota_d[:])
            for t in range(NT):
                rsl = slice(t * 128, (t + 1) * 128)
                dma(out=oma_ts[t][:], in_=oma_d[rsl, :])
                if t == 0:
                    dma(out=h0b[:, P2:EXT], in_=ft_d[0:128, P2:EXT])
                    dma(out=h1b[:, P2:EXT], in_=ft_d[128:256, P2:EXT])

            out_sb = vecp.tile([128, 61], F32, tag="out_sb")
            rs_sl = out_sb[:, 0:NSLOT]
            rs_x = out_sb[:, 28:29]          # second slot of the split pair 0
            pos_sb = out_sb[:, 29:33]
            st_sb = out_sb[:, 33:61]
            cs_sb = vecp.tile([1, EXT], F32, tag="cs_sb")
            pscr = vecp.tile([128, 128], F32, tag="pscr")
            dummy = vecp.tile([128, CHW], BF16, tag="dummy")

            et_tiles = {}          # pair idx -> exp tile handle (per chunk)

            em_ts = {}

            def emit_cekl_act(t, half):
                # two ~2.9us ACT bursts per tile so the PE never idles past
                # the HAM MID window during a CE/KL slot
                if half == 0:
                    nc.scalar.activation(dummy[:, :C], o_ts[t][:], AF.Exp,
                                         scale=1.0,
                                         accum_out=st_sb[:, 0 + t:1 + t])
                    nc.scalar.activation(dummy[:, :C], o_ts[t][:], AF.Exp,
                                         scale=float(1.0 / KL_TEMP),
                                         accum_out=st_sb[:, 4 + t:5 + t])
                else:
                    em_t = scrp.tile([128, C], BF16, tag="em")
                    em_ts[t] = em_t
                    nc.scalar.activation(em_t[:], m_ts[t][:], AF.Exp,
                                         scale=float(1.0 / KL_TEMP),
                                         accum_out=st_sb[:, 8 + t:9 + t])
                    nc.scalar.activation(dummy[:, :C], a_ts[t][:], AF.Exp,
                                         scale=1.0,
                                         accum_out=st_sb[:, 12 + t:13 + t])

            def emit_cekl_dve(t):
                d_t = scrp.tile([128, C], BF16, tag="d")
                nc.vector.tensor_sub(d_t[:], m_ts[t][:], o_ts[t][:])
                nc.vector.scalar_tensor_tensor(
                    out=dummy[:, :C], in0=d_t[:], scalar=1.0, in1=em_ts[t][:],
                    op0=ALU.mult, op1=ALU.mult,
                    accum_out=st_sb[:, 16 + t:17 + t])
                nc.vector.scalar_tensor_tensor(
                    out=dummy[:, :C], in0=iota_t[:], scalar=tg_t[:, t:t + 1],
                    in1=o_ts[t][:], op0=ALU.is_equal, op1=ALU.mult,
                    accum_out=st_sb[:, 20 + t:21 + t])
                nc.vector.scalar_tensor_tensor(
                    out=dummy[:, :C], in0=iota_t[:], scalar=ta_t[:, t:t + 1],
                    in1=a_ts[t][:], op0=ALU.is_equal, op1=ALU.mult,
                    accum_out=st_sb[:, 24 + t:25 + t])

            def emit_gram_exp(idx):
                l, c, a, b = PAIRS[idx]
                w = b - a
                s_l = 128 * l
                gp = gpp.tile([128, CHW], F32, tag="gp")
                if idx == 0:
                    # fast path: close region [0,512) after 3 matmuls so the
                    # very first exp fires as soon as the 512-col DMA lands
                    nc.tensor.matmul(gp[:, 0:512], h0b[:, 0:128],
                                     h0b[:, 0:512], start=True, stop=False,
                                     skip_group_check=True)
                    nc.tensor.matmul(gp[:, 0:128], negidb_t[:], identb_t[:],
                                     start=False, stop=False,
                                     skip_group_check=True)
                    nc.tensor.matmul(gp[:, 0:512], h1b[:, 0:128],
                                     h1b[:, 0:512], start=False, stop=True,
                                     skip_group_check=True)
                    e_t = etp.tile([128, CHW], BF16, tag="et")
                    et_tiles[idx] = e_t
                    nc.scalar.activation(e_t[:, 0:512], gp[:, 0:512], AF.Exp,
                                         scale=float(1.0 / NCE_TEMP),
                                         accum_out=rs_x[:])
                    for half, hb in ((0, h0b), (1, h1b)):
                        for sub in range(512, w, 512):
                            n = min(512, w - sub)
                            nc.tensor.matmul(gp[:, sub:sub + n],
                                             hb[:, 0:128],
                                             hb[:, sub:sub + n],
                                             start=(half == 0),
                                             stop=(half == 1),
                                             skip_group_check=True)
                    nc.scalar.activation(e_t[:, 512:w], gp[:, 512:w], AF.Exp,
                                         scale=float(1.0 / NCE_TEMP),
                                         accum_out=rs_sl[:, 0:1])
                    return
                # gram: all h0 sub-matmuls (start), then h1 (stop), so the
                # stationary operand only swaps once per half
                for half, hb in ((0, h0b), (1, h1b)):
                    for sub in range(0, w, 512):
                        n = min(512, w - sub)
                        nc.tensor.matmul(gp[:, sub:sub + n],
                                         hb[:, s_l:s_l + 128],
                                         hb[:, a + sub:a + sub + n],
                                         start=(half == 0), stop=(half == 1),
                                         skip_group_check=True)
                    if half == 0 and a == s_l:
                        # mask the self-similarity diagonal
                        nc.tensor.matmul(gp[:, 0:128], negidb_t[:], identb_t[:],
                                         start=False, stop=False,
                                         skip_group_check=True)
                # positive-pair logits: diagonal of the d=32 block (l<4 only)
                p0 = s_l + 4096
                if l < 4 and a <= p0 < b:
                    off = p0 - a
                    nc.vector.scalar_tensor_tensor(
                        out=pscr[:], in0=gp[:, off:off + 128], scalar=1.0,
                        in1=ident_t[:], op0=ALU.mult, op1=ALU.mult,
                        accum_out=pos_sb[:, l:l + 1])
                e_t = etp.tile([128, CHW], BF16, tag="et")
                et_tiles[idx] = e_t
                nc.scalar.activation(e_t[:, :w], gp[:, :w], AF.Exp,
                                     scale=float(1.0 / NCE_TEMP),
                                     accum_out=rs_sl[:, idx:idx + 1])

            def emit_colsums(c):
                # mirrored row sums: one 512-wide PSUM accumulator at a time
                for r in range(3 * c, min(3 * c + 3, NREG)):
                    if r not in REGION_PIECES:
                        continue
                    pieces = REGION_PIECES[r]
                    ct = csp.tile([128, RGW], F32, tag="cs")
                    full0 = pieces[0][1] == RGW * r and pieces[0][2] == RGW * (r + 1)
                    if not full0:
                        nc.tensor.matmul(ct[:], onesb_t[:], zerob_t[:],
                                         start=True, stop=False,
                                         skip_group_check=True)
                    for k, (idx, p0_, p1_) in enumerate(pieces):
                        _, _, a, _ = PAIRS[idx]
                        nc.tensor.matmul(
                            ct[:, p0_ - RGW * r:p1_ - RGW * r],
                            onesb_t[:], et_tiles[idx][:, p0_ - a:p1_ - a],
                            start=(k == 0 and full0), stop=(k == len(pieces) - 1),
                            skip_group_check=True)
                    nc.vector.tensor_copy(
                        cs_sb[0:1, RGW * r:RGW * (r + 1)], ct[0:1, :])
                # stream colsums out every second chunk
                if c % 2 == 1 or c == NCHK - 1:
                    lo = RGW * 3 * (c - 1 if c % 2 == 1 else c)
                    hi = min(RGW * 3 * (c + 1), EXT)
                    dma(out=cs_d[0:1, lo:hi], in_=cs_sb[0:1, lo:hi])

            # interleave NCE chunks with CE/KL half-tiles so ACT never starves
            pair_of_chunk = [[i for i, p in enumerate(PAIRS) if p[1] == c]
                             for c in range(NCHK)]
            act_slots = [(t, h) for t in range(NT) for h in (0, 1)]
            dve_slots = list(range(NT))
            for c in range(NCHK):
                # DVE stat work for a tile whose ACT halves are both done
                if dve_slots and len(act_slots) <= 2 * NT - 2 * (
                        dve_slots[0] + 1):
                    emit_cekl_dve(dve_slots.pop(0))
                for k, idx in enumerate(pair_of_chunk[c]):
                    emit_gram_exp(idx)
                    # defer the previous chunk's colsums behind this chunk's
                    # first gram so they never stall the ACT exp pipeline
                    if k == 0 and c >= 1:
                        emit_colsums(c - 1)
                    if c >= 1 and k == len(pair_of_chunk[c]) // 2 and act_slots:
                        emit_cekl_act(*act_slots.pop(0))
                if c >= 1 and act_slots:
                    emit_cekl_act(*act_slots.pop(0))
            emit_colsums(NCHK - 1)
            while dve_slots:
                emit_cekl_dve(dve_slots.pop(0))

            dma(out=out_d[:], in_=out_sb[:])

    nc.compile()
    return nc


_NC = None


def _get_nc():
    global _NC
    if _NC is None:
        _NC = _build_module()
    return _NC


def _prep_inputs(output, target, master_net_pred, feat_pooled,
                 feat_pooled_masked, output_adv, target_adv):
    o = np.ascontiguousarray(np.asarray(output, dtype=np.float32))
    m = np.asarray(master_net_pred, dtype=np.float32)
    a = np.ascontiguousarray(np.asarray(output_adv, dtype=np.float32))
    tg = np.asarray(target).astype(np.int64)
    ta = np.asarray(target_adv).astype(np.int64)
    f0 = np.asarray(feat_pooled, dtype=np.float32)
    f1 = np.asarray(feat_pooled_masked, dtype=np.float32)
    feats = np.concatenate([f0, f1], axis=0)  # [2B, D]
    feats = feats / np.linalg.norm(feats, axis=1, keepdims=True)
    m_bf = m.astype(ml_dtypes.bfloat16)

    in_maps = []
    for cc in range(NCORES):
        sl = slice(cc * RB, (cc + 1) * RB)
        rolled = np.roll(feats, -RB * cc, axis=0)
        ext = np.concatenate([rolled, rolled[:EXT - N2]], axis=0)  # [8704, D]
        ftc = np.ascontiguousarray(ext.T.astype(ml_dtypes.bfloat16))
        tgta = np.zeros((RB, 32), dtype=np.uint8)
        tgc = np.ascontiguousarray(
            tg[sl].reshape(NT, 128).T.astype(np.float32))
        tac = np.ascontiguousarray(
            ta[sl].reshape(NT, 128).T.astype(np.float32))
        tgta[0:128, 0:16] = tgc.view(np.uint8).reshape(128, 16)
        tgta[0:128, 16:32] = tac.view(np.uint8).reshape(128, 16)
        oma = np.concatenate([
            o[sl].view(np.uint8).reshape(RB, -1),
            np.ascontiguousarray(m_bf[sl]).view(np.uint8).reshape(RB, -1),
            a[sl].view(np.uint8).reshape(RB, -1),
            tgta,
        ], axis=1)
        in_maps.append({"oma": np.ascontiguousarray(oma), "ft": ftc})
    return in_maps


def _combine(results):
    S = np.zeros(N2, dtype=np.float64)
    pos_full = np.zeros(N2, dtype=np.float64)
    arp = np.arange(128)
    for cc, rr in enumerate(results):
        rs = rr["out"][:, 0:NSLOT].astype(np.float64)
        rs[:, 0] += rr["out"][:, 28].astype(np.float64)
        cs = rr["cs"].reshape(-1).astype(np.float64)   # [EXT]
        pos = rr["out"][:, 29:33].astype(np.float64)
        for idx, (l, c, a, b) in enumerate(PAIRS):
            rows = (RB * cc + 128 * l + arp) % N2
            np.add.at(S, rows, rs[:, idx])
        gcols = (np.arange(EXT) + RB * cc) % N2
        np.add.at(S, gcols, cs)
        for l in range(4):
            i = RB * cc + 128 * l + arp
            pos_full[i] = pos[:, l]
            pos_full[i + B] = pos[:, l]
    nce_mean = float(np.mean(np.log(S) - pos_full / NCE_TEMP))

    # CE / KL / focal / adv from per-row stats
    sts = [r["out"][:, 33:61] for r in results]
    S1 = np.concatenate([st[:, 0:4].T.reshape(-1) for st in sts])
    ST = np.concatenate([st[:, 4:8].T.reshape(-1) for st in sts])
    SM = np.concatenate([st[:, 8:12].T.reshape(-1) for st in sts])
    SA = np.concatenate([st[:, 12:16].T.reshape(-1) for st in sts])
    PP = np.concatenate([st[:, 16:20].T.reshape(-1) for st in sts])
    GO = np.concatenate([st[:, 20:24].T.reshape(-1) for st in sts])
    GA = np.concatenate([st[:, 24:28].T.reshape(-1) for st in sts])
    S1, ST, SM, SA, PP, GO, GA = (x.astype(np.float64)
                                  for x in (S1, ST, SM, SA, PP, GO, GA))
    ce = np.log(S1) - GO
    adv = np.log(SA) - GA
    kl = PP / (KL_TEMP * SM) - np.log(SM) + np.log(ST)
    pt = np.exp(-ce)
    gamma = np.where(pt < 0.2, 5.0, np.where(pt < 0.5, 3.0, 1.0))
    foc = ((1.0 - pt) ** gamma) * ce
    loss = (KL_INTERP * KL_TEMP * KL_TEMP) * np.mean(kl) / C \
        + (1.0 - KL_INTERP) * np.mean(ce) + nce_mean \
        + np.mean(foc) + np.mean(adv)
    return np.asarray([loss], dtype=np.float32)


def kernel(**inputs):
    in_maps = _prep_inputs(**inputs)
    out = run_bass_kernel_spmd(_get_nc(), in_maps,
                               core_ids=list(range(NCORES)))
    return _combine(out.results)


if __name__ == "__main__":
    rng = np.random.default_rng(0)
    ins = {
        "output": rng.standard_normal((B, C), dtype=np.float32),
        "target": rng.integers(0, C, size=(B,)),
        "master_net_pred": rng.standard_normal((B, C), dtype=np.float32),
        "feat_pooled": rng.standard_normal((B, D), dtype=np.float32),
        "feat_pooled_masked": rng.standard_normal((B, D), dtype=np.float32),
        "output_adv": rng.standard_normal((B, C), dtype=np.float32),
        "target_adv": rng.integers(0, C, size=(B,)),
    }
    print(kernel(**ins))



# revision 9
# speedup vs baseline: 1.2060x; 1.2060x over previous
"""Trainium2 Bass kernel for the combined loss (KL + CE + InfoNCE + focal + adv CE).

v3 strategy (8 NeuronCores, data-parallel over the batch):
  - InfoNCE exploits Gram symmetry: each block-row computes circulant
    distances 0..32 only; the mirrored distances come from PE ones-matmul
    column sums of the exp tiles (as in v2).
  - Gram matmuls run in fp8(e4m3) DoubleRow mode: K=256 contraction in one
    pass at 2 cols/cycle.  Features are normalized, pre-scaled by
    sqrt(log2e/(32*T)) and packed [128, 2, EXT] on the host, so the PSUM
    values are y with exp(G/T) = 2^(32y).
  - Gram exp runs on ACT (scale=1/(s2*T)) with row-sum accumulators.
  - CE/KL/adv exp row-sums are split between ACT and a custom 2-instruction
    DVE exp pipeline (quartic poly in 2^y then 5 squarings, scale folded
    into the coefficients), balancing the two engines.
  - o/m/a ship as bf16; targets are gathered on the host (GO/GA stay host-side).
  - Colsum PSUM rows copy out through DVE; per-sample epilogue on host.
"""

import numpy as np
import ml_dtypes
from operator import add as _add

import concourse.bacc as bacc
import concourse.tile as tile
from concourse import mybir
from concourse.bass_utils import run_bass_kernel_spmd

import concourse.dve_ops as DO
from concourse.dve_spec import (Spec, Src0, C0 as _C0, C1 as _C1, C2 as _C2,
                                C3 as _C3, Zero, One, lower as _dve_lower,
                                sq as _sq, _spill_c3_to_src1, _has_src1)
from concourse.dve_uop import DveOpSpec

F32 = mybir.dt.float32
BF16 = mybir.dt.bfloat16
FP8 = mybir.dt.float8e4
AF = mybir.ActivationFunctionType
ALU = mybir.AluOpType
DR = mybir.MatmulPerfMode.DoubleRow

NCORES = 8
B, C, D = 4096, 1000, 256
RB = B // NCORES          # 512 rows of the [B, C] tensors per core
NT = RB // 128            # 4 row-tiles per core
N2 = 2 * B                # 8192 infoNCE rows
NDIST = 33                # circulant distances d = 0..32 per block-row
SPAN = NDIST * 128        # 4224 columns per block-row sweep
L_ROWS = [0, 1, 2, 3, 32, 33, 34, 35]   # local block-row indices (all cores)
EXT = 35 * 128 + SPAN     # 8704 extended local columns
CHW = 1536                # gram/exp chunk width (3 PSUM banks)
NCHK = (EXT + CHW - 1) // CHW           # 6 chunks
RGW = 512                 # colsum accumulation region width (1 PSUM bank)
NREG = EXT // RGW         # 17 regions

KL_TEMP = 4.0
KL_INTERP = 0.5
NCE_TEMP = 0.07
LOG2E = float(np.log2(np.e))
S2 = LOG2E / (32.0 * NCE_TEMP)      # feature pre-scale^2; y = S2 * cos_sim
ACT_SCALE = 1.0 / (S2 * NCE_TEMP)   # exp(ACT_SCALE * y) = exp(G/T)
NEG_DIAG = -1.875                   # diag mask add (y_diag ~ -1.231)

# exp2 quartic: 2^y ~ c0*(1 + y(b1 + y(b2 + y(b3 + y*b4)))) on [-1.30, 0.67]
PB1, PB2, PB3, PB4 = 0.69336677, 0.24124203, 0.05543758, 0.00758271
PC0_32 = 0.9979927195289331         # c0^32

OMA_W = 6016              # o|m|a bf16 (2000 each) + 16 pad


def _register_dve(name, spec):
    if name in DO._SUB_OPCODE_FOR_NAME:
        return next(o for o in DO.OPS if o.name == name)
    op = DO.DveOp(name, spec, subdim=False, uops_sha={})
    DO.OPS.append(op)
    DO._SUB_OPCODE_FOR_NAME[name] = DO._CUSTOM_DVE_ROW_BASE + len(DO.OPS) - 1
    DO.CUSTOM_DVE_SPECS[name] = spec
    for ver in ("v3", "v4"):
        s = DveOpSpec(name=name, opcode=DO.get_dve_sub_opcode(name),
                      uops=_dve_lower(spec, ver=ver), rd1_en=_has_src1(spec))
        op.uops_sha[ver] = s.sha(ver)
    return op


def _ref_exp2pa(in0, in1, s0, s1, imm2):
    x = in0.astype(np.float32)
    return (1.0 + x * (s0 + x * (s1 + x * (imm2 + x * in1)))).astype(np.float32)


def _ref_exp2pb(in0, in1, s0, s1, imm2):
    b = ((in0.astype(np.float64) ** 32) * s0).astype(np.float32)
    return b, b.reshape(b.shape[0], -1).sum(axis=-1, keepdims=True).astype(
        np.float32)


EXP2PA = _register_dve("EXP2PA_ANT", Spec(
    body=_spill_c3_to_src1(
        One + Src0 * (_C0 + Src0 * (_C1 + Src0 * (_C2 + Src0 * _C3)))),
    reference=_ref_exp2pa))
EXP2PB = _register_dve("EXP2PB_ANT", Spec(
    body=_sq(_sq(_sq(_sq(_sq(Src0))))) * _C0, accum=_add, accum_init=Zero,
    reference=_ref_exp2pb))


def _poly_coefs(scale):
    """Fold an input pre-scale into the exp2 quartic: R(s*x) coefficients."""
    s = float(scale)
    return PB1 * s, PB2 * s * s, PB3 * s ** 3, PB4 * s ** 4


def _pair_table():
    """(l, c, a, b) for every (block-row, chunk) intersection, chunk-major."""
    pairs = []
    for c in range(NCHK):
        c0, c1 = CHW * c, min(CHW * (c + 1), EXT)
        for l in L_ROWS:
            s, e = 128 * l, 128 * l + SPAN
            a, b = max(c0, s), min(c1, e)
            if b > a:
                pairs.append((l, c, a, b))
    return pairs


PAIRS = _pair_table()
NSLOT = len(PAIRS)        # 28


def _region_pieces():
    """region -> list of (pair_idx, p0, p1) colsum pieces; a full-covering
    piece (if any) is moved to the front so no zero-init matmul is needed."""
    reg = {}
    for idx, (l, c, a, b) in enumerate(PAIRS):
        a2 = max(a, 128 * l + 128)        # exclude d=0 (diagonal block)
        b2 = min(b, 128 * l + 4096)       # exclude d=32 (rowsum-only block)
        if b2 <= a2:
            continue
        r0, r1 = a2 // RGW, (b2 - 1) // RGW
        for r in range(r0, r1 + 1):
            p0, p1 = max(a2, RGW * r), min(b2, RGW * (r + 1))
            reg.setdefault(r, []).append((idx, p0, p1))
    for r, pieces in reg.items():
        full = [k for k, (_, p0, p1) in enumerate(pieces)
                if p0 == RGW * r and p1 == RGW * (r + 1)]
        if full:
            k = full[0]
            pieces.insert(0, pieces.pop(k))
    return reg


REGION_PIECES = _region_pieces()

# CE/KL exp units: (tile, kind); kind 0=exp(o), 1=exp(o/4), 2=exp(m/4)->em,
# 3=exp(a).  Stat slot column = 33 + kind*4 + tile (S1|ST|SM|SA), PP at 49+t.
CEKL_ON_ACT = {(0, 0), (0, 2), (1, 2), (2, 2), (3, 2)}   # 5 units on ACT
OUT_W = 53


def _build_module():
    nc = bacc.Bacc("TRN2", target_bir_lowering=False, debug=False)

    oma_d = nc.dram_tensor("oma", [RB, OMA_W], mybir.dt.uint8,
                           kind="ExternalInput")
    hp_d = nc.dram_tensor("hp", [128, 2 * EXT], mybir.dt.uint8,
                          kind="ExternalInput")
    out_d = nc.dram_tensor("out", [128, OUT_W], F32, kind="ExternalOutput")
    cs_d = nc.dram_tensor("cs", [1, EXT], F32, kind="ExternalOutput")

    # packed constants: ident f32 | identb bf16 | negidb bf16 | onesb bf16 |
    # zerob bf16 | poly consts f32 (b4 variants)
    cpack = np.concatenate([
        np.eye(128, dtype=np.float32).view(np.uint8).reshape(128, -1),
        np.eye(128).astype(ml_dtypes.bfloat16).view(np.uint8).reshape(128, -1),
        (NEG_DIAG * np.eye(128)).astype(ml_dtypes.bfloat16).view(
            np.uint8).reshape(128, -1),
        np.ones((128, 128)).astype(ml_dtypes.bfloat16).view(
            np.uint8).reshape(128, -1),
        np.zeros((128, RGW)).astype(ml_dtypes.bfloat16).view(
            np.uint8).reshape(128, -1),
        np.tile(np.array([[_poly_coefs(1.0)[3],
                           _poly_coefs(LOG2E / 32.0)[3],
                           _poly_coefs(LOG2E / 128.0)[3]]], np.float32),
                (128, 1)).view(np.uint8).reshape(128, -1),
    ], axis=1)
    cpack_d = nc.inline_tensor(cpack, "cpack_c")

    with tile.TileContext(nc) as tc:
        with (
            tc.tile_pool(name="persist", bufs=1) as persist,
            tc.tile_pool(name="io", bufs=1) as iop,
            tc.tile_pool(name="em", bufs=4) as emp,
            tc.tile_pool(name="scr", bufs=2) as scrp,
            tc.tile_pool(name="qt", bufs=2) as qtp,
            tc.tile_pool(name="et", bufs=16) as etp,
            tc.tile_pool(name="vec", bufs=1) as vecp,
            tc.tile_pool(name="gp", bufs=2, space="PSUM") as gpp,
            tc.tile_pool(name="cs", bufs=2, space="PSUM") as csp,
        ):
            dma = nc.sync.dma_start

            cpack_t = persist.tile([128, cpack.shape[1]], mybir.dt.uint8,
                                   tag="cpack")
            ident_t = cpack_t[:, 0:512].bitcast(F32)
            identb_t = cpack_t[:, 512:768].bitcast(BF16)
            negidb_t = cpack_t[:, 768:1024].bitcast(BF16)
            onesb_t = cpack_t[:, 1024:1280].bitcast(BF16)
            zerob_t = cpack_t[:, 1280:1280 + 2 * RGW].bitcast(BF16)
            b4c_t = cpack_t[:, 1280 + 2 * RGW:1280 + 2 * RGW + 12].bitcast(F32)

            hp8 = persist.tile([128, 2, EXT], FP8, tag="hp8")
            hp8u = hp8.bitcast(mybir.dt.uint8)

            oma_ts, o_ts, m_ts, a_ts = [], [], [], []
            for t in range(NT):
                oma_t = iop.tile([128, OMA_W], mybir.dt.uint8, tag=f"oma{t}")
                oma_ts.append(oma_t)
                o_ts.append(oma_t[:, 0:2000].bitcast(BF16))
                m_ts.append(oma_t[:, 2000:4000].bitcast(BF16))
                a_ts.append(oma_t[:, 4000:6000].bitcast(BF16))

            # progressive feature pieces so the first gram starts early
            P0, P1, P2 = 640, 1536, 4608
            dma(out=cpack_t[:], in_=cpack_d[:])

            def dma_hp(lo, hi):
                for j in (0, 1):
                    dma(out=hp8u[:, j:j + 1, lo:hi],
                        in_=hp_d[:, j * EXT + lo:j * EXT + hi])

            dma_hp(0, P0)
            dma_hp(P0, P1)
            dma(out=oma_ts[0][:], in_=oma_d[0:128, :])
            dma_hp(P1, P2)
            dma(out=oma_ts[1][:], in_=oma_d[128:256, :])
            dma_hp(P2, EXT)
            dma(out=oma_ts[2][:], in_=oma_d[256:384, :])
            dma(out=oma_ts[3][:], in_=oma_d[384:512, :])

            out_sb = vecp.tile([128, OUT_W], F32, tag="out_sb")
            rs_sl = out_sb[:, 0:NSLOT]
            rs_x = out_sb[:, 28:29]          # second slot of the split pair 0
            pos_sb = out_sb[:, 29:33]
            st_sb = out_sb[:, 33:OUT_W]
            cs_sb = vecp.tile([1, EXT], F32, tag="cs_sb")
            pscr = vecp.tile([128, 128], F32, tag="pscr")
            dummy_a = vecp.tile([128, 1000], BF16, tag="dummy_a")  # ACT-only
            dummy_v = vecp.tile([128, 1000], BF16, tag="dummy_v")  # DVE-only

            et_tiles = {}
            em_ts = {}

            def emit_gram(idx):
                l, c, a, b = PAIRS[idx]
                w = b - a
                s_l = 128 * l
                gp = gpp.tile([128, CHW], F32, tag="gp")
                et_tiles[idx] = (gp, None)
                lhsT = hp8[:, :, s_l:s_l + 128]
                for sub in range(0, w, 512):
                    n = min(512, w - sub)
                    d0 = a + sub <= s_l < a + sub + n
                    nc.tensor.matmul(gp[:, sub:sub + n], lhsT,
                                     hp8[:, :, a + sub:a + sub + n],
                                     perf_mode=DR, start=True, stop=not d0,
                                     skip_group_check=True)
                    if d0:
                        off = s_l - a
                        nc.tensor.matmul(gp[:, off:off + 128], negidb_t[:],
                                         identb_t[:], start=False, stop=True,
                                         skip_group_check=True)
                return gp

            def emit_pos(idx, gp):
                l, c, a, b = PAIRS[idx]
                p0 = 128 * l + 4096
                if l < 4 and a <= p0 < b:
                    off = p0 - a
                    nc.vector.scalar_tensor_tensor(
                        out=pscr[:], in0=gp[:, off:off + 128], scalar=1.0,
                        in1=ident_t[:], op0=ALU.mult, op1=ALU.mult,
                        accum_out=pos_sb[:, l:l + 1])

            def emit_exp_act(idx, gp, split=False):
                l, c, a, b = PAIRS[idx]
                w = b - a
                e_t = etp.tile([128, CHW], BF16, tag="et")
                et_tiles[idx] = (gp, e_t)
                if split:
                    nc.scalar.activation(e_t[:, 0:512], gp[:, 0:512], AF.Exp,
                                         scale=ACT_SCALE, accum_out=rs_x[:])
                    nc.scalar.activation(e_t[:, 512:w], gp[:, 512:w], AF.Exp,
                                         scale=ACT_SCALE,
                                         accum_out=rs_sl[:, idx:idx + 1])
                else:
                    nc.scalar.activation(e_t[:, :w], gp[:, :w], AF.Exp,
                                         scale=ACT_SCALE,
                                         accum_out=rs_sl[:, idx:idx + 1])

            def emit_cekl_unit(t, kind):
                src = (o_ts[t], o_ts[t], m_ts[t], a_ts[t])[kind]
                slot = st_sb[:, kind * 4 + t:kind * 4 + t + 1]
                on_act = (t, kind) in CEKL_ON_ACT
                if kind == 2:
                    em_t = emp.tile([128, 1000], BF16, tag="em")
                    em_ts[t] = em_t
                    dst = em_t[:]
                else:
                    dst = (dummy_a if on_act else dummy_v)[:, 0:1000]
                if on_act:
                    scale = 1.0 if kind in (0, 3) else 0.25
                    nc.scalar.activation(dst, src[:], AF.Exp, scale=scale,
                                         accum_out=slot)
                else:
                    s = LOG2E / 32.0 if kind in (0, 3) else LOG2E / 128.0
                    b1, b2, b3, _ = _poly_coefs(s)
                    b4col = 1 if kind in (0, 3) else 2
                    q_t = qtp.tile([128, 1000], F32, tag="q")
                    nc.vector._custom_dve(
                        EXP2PA, out=q_t[:], in0=src[:],
                        in1=b4c_t[:, b4col:b4col + 1], s0=b1, s1=b2, imm2=b3)
                    nc.vector._custom_dve(
                        EXP2PB, out=dst, in0=q_t[:], s0=PC0_32,
                        accum_out=slot)

            def emit_cekl_pp(t):
                d_t = scrp.tile([128, 1000], BF16, tag="d")
                nc.vector.tensor_sub(d_t[:], m_ts[t][:], o_ts[t][:])
                nc.vector.scalar_tensor_tensor(
                    out=dummy_v[:, 0:1000], in0=d_t[:], scalar=1.0,
                    in1=em_ts[t][:], op0=ALU.mult, op1=ALU.mult,
                    accum_out=st_sb[:, 16 + t:17 + t])

            def emit_colsums(c):
                for r in range(3 * c, min(3 * c + 3, NREG)):
                    if r not in REGION_PIECES:
                        continue
                    pieces = REGION_PIECES[r]
                    ct = csp.tile([128, RGW], F32, tag="cs")
                    full0 = (pieces[0][1] == RGW * r
                             and pieces[0][2] == RGW * (r + 1))
                    if not full0:
                        nc.tensor.matmul(ct[:], onesb_t[:], zerob_t[:],
                                         start=True, stop=False,
                                         skip_group_check=True)
                    for k, (idx, p0_, p1_) in enumerate(pieces):
                        _, _, a, _ = PAIRS[idx]
                        e_t = et_tiles[idx][1]
                        nc.tensor.matmul(
                            ct[:, p0_ - RGW * r:p1_ - RGW * r],
                            onesb_t[:], e_t[:, p0_ - a:p1_ - a],
                            start=(k == 0 and full0),
                            stop=(k == len(pieces) - 1),
                            skip_group_check=True)
                    nc.vector.tensor_copy(
                        cs_sb[0:1, RGW * r:RGW * (r + 1)], ct[0:1, :])
                if c % 2 == 1 or c == NCHK - 1:
                    lo = RGW * 3 * (c - 1 if c % 2 == 1 else c)
                    hi = min(RGW * 3 * (c + 1), EXT)
                    dma(out=cs_d[0:1, lo:hi], in_=cs_sb[0:1, lo:hi])

            # unit emission order: DVE units early (DMA-dependent only) in
            # tile order matching DMA arrival; ACT units fill gram gaps.
            dve_units = [(0, 1), (0, 3), (1, 1), (1, 0), (1, 3), (2, 1),
                         (2, 0), (2, 3), (3, 1), (3, 0), (3, 3)]
            act_units = [(0, 0), (1, 2), (2, 2), (3, 2)]
            pair_of_chunk = [[i for i, p in enumerate(PAIRS) if p[1] == c]
                             for c in range(NCHK)]
            pp_done = 0
            for c in range(NCHK):
                for k, idx in enumerate(pair_of_chunk[c]):
                    gp = emit_gram(idx)
                    if k == 0 and c >= 1:
                        emit_colsums(c - 1)
                    emit_pos(idx, gp)
                    emit_exp_act(idx, gp, split=(idx == 0))
                    if idx == 0:
                        # m-exp for tile 0 early, then DVE filler units
                        emit_cekl_unit(0, 2)
                        for _ in range(2):
                            if dve_units:
                                emit_cekl_unit(*dve_units.pop(0))
                    elif k == len(pair_of_chunk[c]) // 2 and c >= 1:
                        if act_units:
                            emit_cekl_unit(*act_units.pop(0))
                        if dve_units:
                            emit_cekl_unit(*dve_units.pop(0))
                if c >= 1:
                    if act_units:
                        emit_cekl_unit(*act_units.pop(0))
                    if dve_units:
                        emit_cekl_unit(*dve_units.pop(0))
                    if pp_done < NT and pp_done in em_ts:
                        emit_cekl_pp(pp_done)
                        pp_done += 1
            emit_colsums(NCHK - 1)
            for u in act_units:
                emit_cekl_unit(*u)
            for u in dve_units:
                emit_cekl_unit(*u)
            while pp_done < NT:
                if pp_done in em_ts:
                    emit_cekl_pp(pp_done)
                pp_done += 1

            dma(out=out_d[:], in_=out_sb[:])

    nc.compile()
    return nc


_NC = None


def _get_nc():
    global _NC
    if _NC is None:
        _NC = _build_module()
    return _NC


_HOST = {}


def _prep_inputs(output, target, master_net_pred, feat_pooled,
                 feat_pooled_masked, output_adv, target_adv):
    o = np.asarray(output, dtype=np.float32)
    m = np.asarray(master_net_pred, dtype=np.float32)
    a = np.asarray(output_adv, dtype=np.float32)
    tg = np.asarray(target).astype(np.int64)
    ta = np.asarray(target_adv).astype(np.int64)
    f0 = np.asarray(feat_pooled, dtype=np.float32)
    f1 = np.asarray(feat_pooled_masked, dtype=np.float32)
    feats = np.concatenate([f0, f1], axis=0)  # [2B, D]
    feats = feats / np.linalg.norm(feats, axis=1, keepdims=True)
    feats = feats * np.float32(np.sqrt(S2))

    _HOST["GO"] = np.take_along_axis(o, tg[:, None], axis=1)[:, 0]
    _HOST["GA"] = np.take_along_axis(a, ta[:, None], axis=1)[:, 0]

    o_bf = o.astype(ml_dtypes.bfloat16)
    m_bf = m.astype(ml_dtypes.bfloat16)
    a_bf = a.astype(ml_dtypes.bfloat16)

    in_maps = []
    for cc in range(NCORES):
        sl = slice(cc * RB, (cc + 1) * RB)
        rolled = np.roll(feats, -RB * cc, axis=0)
        ext = np.concatenate([rolled, rolled[:EXT - N2]], axis=0)  # [8704, D]
        f8 = np.ascontiguousarray(ext.T).astype(ml_dtypes.float8_e4m3)
        hp = np.concatenate([f8[0:128], f8[128:256]], axis=1)  # [128, 2*EXT]
        oma = np.zeros((RB, OMA_W), dtype=np.uint8)
        oma[:, 0:2000] = np.ascontiguousarray(o_bf[sl]).view(np.uint8)
        oma[:, 2000:4000] = np.ascontiguousarray(m_bf[sl]).view(np.uint8)
        oma[:, 4000:6000] = np.ascontiguousarray(a_bf[sl]).view(np.uint8)
        in_maps.append({"oma": oma, "hp": hp.view(np.uint8)})
    return in_maps


def _combine(results):
    S = np.zeros(N2, dtype=np.float64)
    pos_full = np.zeros(N2, dtype=np.float64)
    arp = np.arange(128)
    for cc, rr in enumerate(results):
        rs = rr["out"][:, 0:NSLOT].astype(np.float64)
        rs[:, 0] += rr["out"][:, 28].astype(np.float64)
        cs = rr["cs"].reshape(-1).astype(np.float64)   # [EXT]
        pos = rr["out"][:, 29:33].astype(np.float64)
        for idx, (l, c, a, b) in enumerate(PAIRS):
            rows = (RB * cc + 128 * l + arp) % N2
            np.add.at(S, rows, rs[:, idx])
        gcols = (np.arange(EXT) + RB * cc) % N2
        np.add.at(S, gcols, cs)
        for l in range(4):
            i = RB * cc + 128 * l + arp
            pos_full[i] = pos[:, l]
            pos_full[i + B] = pos[:, l]
    pos_logit = pos_full * ACT_SCALE
    nce_mean = float(np.mean(np.log(S) - pos_logit))

    # CE / KL / focal / adv from per-row stats
    sts = [r["out"][:, 33:OUT_W] for r in results]
    S1 = np.concatenate([st[:, 0:4].T.reshape(-1) for st in sts])
    ST = np.concatenate([st[:, 4:8].T.reshape(-1) for st in sts])
    SM = np.concatenate([st[:, 8:12].T.reshape(-1) for st in sts])
    SA = np.concatenate([st[:, 12:16].T.reshape(-1) for st in sts])
    PP = np.concatenate([st[:, 16:20].T.reshape(-1) for st in sts])
    S1, ST, SM, SA, PP = (x.astype(np.float64)
                          for x in (S1, ST, SM, SA, PP))
    GO = _HOST["GO"].astype(np.float64)
    GA = _HOST["GA"].astype(np.float64)
    ce = np.log(S1) - GO
    adv = np.log(SA) - GA
    kl = PP / (KL_TEMP * SM) - np.log(SM) + np.log(ST)
    pt = np.exp(-ce)
    gamma = np.where(pt < 0.2, 5.0, np.where(pt < 0.5, 3.0, 1.0))
    foc = ((1.0 - pt) ** gamma) * ce
    loss = (KL_INTERP * KL_TEMP * KL_TEMP) * np.mean(kl) / C \
        + (1.0 - KL_INTERP) * np.mean(ce) + nce_mean \
        + np.mean(foc) + np.mean(adv)
    return np.asarray([loss], dtype=np.float32)


def kernel(**inputs):
    in_maps = _prep_inputs(**inputs)
    out = run_bass_kernel_spmd(_get_nc(), in_maps,
                               core_ids=list(range(NCORES)))
    return _combine(out.results)


if __name__ == "__main__":
    rng = np.random.default_rng(0)
    ins = {
        "output": rng.standard_normal((B, C), dtype=np.float32),
        "target": rng.integers(0, C, size=(B,)),
        "master_net_pred": rng.standard_normal((B, C), dtype=np.float32),
        "feat_pooled": rng.standard_normal((B, D), dtype=np.float32),
        "feat_pooled_masked": rng.standard_normal((B, D), dtype=np.float32),
        "output_adv": rng.standard_normal((B, C), dtype=np.float32),
        "target_adv": rng.integers(0, C, size=(B,)),
    }
    print(kernel(**ins))


# revision 12
# speedup vs baseline: 1.2405x; 1.0286x over previous
"""Trainium2 Bass kernel for the combined loss (KL + CE + InfoNCE + focal + adv CE).

v3 strategy (8 NeuronCores, data-parallel over the batch):
  - InfoNCE exploits Gram symmetry: each block-row computes circulant
    distances 0..32 only; the mirrored distances come from PE ones-matmul
    column sums of the exp tiles (as in v2).
  - Gram matmuls run in fp8(e4m3) DoubleRow mode: K=256 contraction in one
    pass at 2 cols/cycle.  Features are normalized, pre-scaled by
    sqrt(log2e/(32*T)) and packed [128, 2, EXT] on the host, so the PSUM
    values are y with exp(G/T) = 2^(32y).
  - Gram exp runs on ACT (scale=1/(s2*T)) with row-sum accumulators.
  - CE/KL/adv exp row-sums are split between ACT and a custom 2-instruction
    DVE exp pipeline (quartic poly in 2^y then 5 squarings, scale folded
    into the coefficients), balancing the two engines.
  - o/m/a ship as bf16; targets are gathered on the host (GO/GA stay host-side).
  - Colsum PSUM rows copy out through DVE; per-sample epilogue on host.
"""

import numpy as np
import ml_dtypes
from operator import add as _add

import concourse.bacc as bacc
import concourse.tile as tile
from concourse import mybir
from concourse.bass_utils import run_bass_kernel_spmd

import concourse.dve_ops as DO
from concourse.dve_spec import (Spec, Src0, C0 as _C0, C1 as _C1, C2 as _C2,
                                C3 as _C3, Zero, One, lower as _dve_lower,
                                sq as _sq, _spill_c3_to_src1, _has_src1)
from concourse.dve_uop import DveOpSpec

F32 = mybir.dt.float32
BF16 = mybir.dt.bfloat16
FP8 = mybir.dt.float8e4
AF = mybir.ActivationFunctionType
ALU = mybir.AluOpType
DR = mybir.MatmulPerfMode.DoubleRow

NCORES = 8
B, C, D = 4096, 1000, 256
RB = B // NCORES          # 512 rows of the [B, C] tensors per core
NT = RB // 128            # 4 row-tiles per core
N2 = 2 * B                # 8192 infoNCE rows
NDIST = 33                # circulant distances d = 0..32 per block-row
SPAN = NDIST * 128        # 4224 columns per block-row sweep
L_ROWS = [0, 1, 2, 3, 32, 33, 34, 35]   # local block-row indices (all cores)
EXT = 35 * 128 + SPAN     # 8704 extended local columns
CHW = 1536                # gram/exp chunk width (3 PSUM banks)
NCHK = (EXT + CHW - 1) // CHW           # 6 chunks
RGW = 512                 # colsum accumulation region width (1 PSUM bank)
NREG = EXT // RGW         # 17 regions

KL_TEMP = 4.0
KL_INTERP = 0.5
NCE_TEMP = 0.07
LOG2E = float(np.log2(np.e))
S2 = LOG2E / (32.0 * NCE_TEMP)      # feature pre-scale^2; y = S2 * cos_sim
ACT_SCALE = 1.0 / (S2 * NCE_TEMP)   # exp(ACT_SCALE * y) = exp(G/T)
NEG_DIAG = -1.875                   # diag mask add (y_diag ~ -1.231)

# exp2 quartic: 2^y ~ c0*(1 + y(b1 + y(b2 + y(b3 + y*b4)))) on [-1.30, 0.67]
PB1, PB2, PB3, PB4 = 0.69336677, 0.24124203, 0.05543758, 0.00758271
PC0_32 = 0.9979927195289331         # c0^32

OMA_W = 6016              # o|m|a bf16 (2000 each) + 16 pad


def _register_dve(name, spec):
    if name in DO._SUB_OPCODE_FOR_NAME:
        return next(o for o in DO.OPS if o.name == name)
    op = DO.DveOp(name, spec, subdim=False, uops_sha={})
    DO.OPS.append(op)
    DO._SUB_OPCODE_FOR_NAME[name] = DO._CUSTOM_DVE_ROW_BASE + len(DO.OPS) - 1
    DO.CUSTOM_DVE_SPECS[name] = spec
    for ver in ("v3", "v4"):
        s = DveOpSpec(name=name, opcode=DO.get_dve_sub_opcode(name),
                      uops=_dve_lower(spec, ver=ver), rd1_en=_has_src1(spec))
        op.uops_sha[ver] = s.sha(ver)
    return op


def _ref_exp2pa(in0, in1, s0, s1, imm2):
    x = in0.astype(np.float32)
    return (1.0 + x * (s0 + x * (s1 + x * (imm2 + x * in1)))).astype(np.float32)


def _ref_exp2pb(in0, in1, s0, s1, imm2):
    b = ((in0.astype(np.float64) ** 32) * s0).astype(np.float32)
    return b, b.reshape(b.shape[0], -1).sum(axis=-1, keepdims=True).astype(
        np.float32)


EXP2PA = _register_dve("EXP2PA_ANT", Spec(
    body=_spill_c3_to_src1(
        One + Src0 * (_C0 + Src0 * (_C1 + Src0 * (_C2 + Src0 * _C3)))),
    reference=_ref_exp2pa))
EXP2PB = _register_dve("EXP2PB_ANT", Spec(
    body=_sq(_sq(_sq(_sq(_sq(Src0))))) * _C0, accum=_add, accum_init=Zero,
    reference=_ref_exp2pb))


def _poly_coefs(scale):
    """Fold an input pre-scale into the exp2 quartic: R(s*x) coefficients."""
    s = float(scale)
    return PB1 * s, PB2 * s * s, PB3 * s ** 3, PB4 * s ** 4


def _pair_table():
    """(l, c, a, b) for every (block-row, chunk) intersection, chunk-major."""
    pairs = []
    for c in range(NCHK):
        c0, c1 = CHW * c, min(CHW * (c + 1), EXT)
        for l in L_ROWS:
            s, e = 128 * l, 128 * l + SPAN
            a, b = max(c0, s), min(c1, e)
            if b > a:
                pairs.append((l, c, a, b))
    return pairs


PAIRS = _pair_table()
NSLOT = len(PAIRS)        # 28


def _region_pieces():
    """region -> list of (pair_idx, p0, p1) colsum pieces; a full-covering
    piece (if any) is moved to the front so no zero-init matmul is needed."""
    reg = {}
    for idx, (l, c, a, b) in enumerate(PAIRS):
        a2 = max(a, 128 * l + 128)        # exclude d=0 (diagonal block)
        b2 = min(b, 128 * l + 4096)       # exclude d=32 (rowsum-only block)
        if b2 <= a2:
            continue
        r0, r1 = a2 // RGW, (b2 - 1) // RGW
        for r in range(r0, r1 + 1):
            p0, p1 = max(a2, RGW * r), min(b2, RGW * (r + 1))
            reg.setdefault(r, []).append((idx, p0, p1))
    for r, pieces in reg.items():
        full = [k for k, (_, p0, p1) in enumerate(pieces)
                if p0 == RGW * r and p1 == RGW * (r + 1)]
        if full:
            k = full[0]
            pieces.insert(0, pieces.pop(k))
    return reg


REGION_PIECES = _region_pieces()

# CE/KL exp units: (tile, kind); kind 0=exp(o), 1=exp(o/4), 2=exp(m/4)->em,
# 3=exp(a).  Stat slot column = 33 + kind*4 + tile (S1|ST|SM|SA), PP at 49+t.
CEKL_ON_ACT = {(0, 0), (0, 2), (1, 2), (2, 2), (3, 2)}   # 5 units on ACT
OUT_W = 53


def _build_module():
    nc = bacc.Bacc("TRN2", target_bir_lowering=False, debug=False)

    oma_d = nc.dram_tensor("oma", [RB, OMA_W], mybir.dt.uint8,
                           kind="ExternalInput")
    hp_d = nc.dram_tensor("hp", [128, 2 * EXT], mybir.dt.uint8,
                          kind="ExternalInput")
    out_d = nc.dram_tensor("out", [128, OUT_W], F32, kind="ExternalOutput")
    cs_d = nc.dram_tensor("cs", [1, EXT], F32, kind="ExternalOutput")

    # packed constants: ident f32 | identb bf16 | negidb bf16 | onesb bf16 |
    # zerob bf16 | poly consts f32 (b4 variants)
    cpack = np.concatenate([
        np.eye(128, dtype=np.float32).view(np.uint8).reshape(128, -1),
        np.eye(128).astype(ml_dtypes.bfloat16).view(np.uint8).reshape(128, -1),
        (NEG_DIAG * np.eye(128)).astype(ml_dtypes.bfloat16).view(
            np.uint8).reshape(128, -1),
        np.ones((128, 128)).astype(ml_dtypes.bfloat16).view(
            np.uint8).reshape(128, -1),
        np.zeros((128, RGW)).astype(ml_dtypes.bfloat16).view(
            np.uint8).reshape(128, -1),
        np.tile(np.array([[_poly_coefs(1.0)[3],
                           _poly_coefs(LOG2E / 32.0)[3],
                           _poly_coefs(LOG2E / 128.0)[3]]], np.float32),
                (128, 1)).view(np.uint8).reshape(128, -1),
    ], axis=1)
    cpack_d = nc.inline_tensor(cpack, "cpack_c")

    with tile.TileContext(nc) as tc:
        with (
            tc.tile_pool(name="persist", bufs=1) as persist,
            tc.tile_pool(name="io", bufs=1) as iop,
            tc.tile_pool(name="em", bufs=4) as emp,
            tc.tile_pool(name="scr", bufs=2) as scrp,
            tc.tile_pool(name="qt", bufs=2) as qtp,
            tc.tile_pool(name="et", bufs=16) as etp,
            tc.tile_pool(name="vec", bufs=1) as vecp,
            tc.tile_pool(name="gp", bufs=2, space="PSUM") as gpp,
            tc.tile_pool(name="cs", bufs=2, space="PSUM") as csp,
        ):
            dma = nc.sync.dma_start

            cpack_t = persist.tile([128, cpack.shape[1]], mybir.dt.uint8,
                                   tag="cpack")
            ident_t = cpack_t[:, 0:512].bitcast(F32)
            identb_t = cpack_t[:, 512:768].bitcast(BF16)
            negidb_t = cpack_t[:, 768:1024].bitcast(BF16)
            onesb_t = cpack_t[:, 1024:1280].bitcast(BF16)
            zerob_t = cpack_t[:, 1280:1280 + 2 * RGW].bitcast(BF16)
            b4c_t = cpack_t[:, 1280 + 2 * RGW:1280 + 2 * RGW + 12].bitcast(F32)

            hp8 = persist.tile([128, 2, EXT], FP8, tag="hp8")
            hp8u = hp8.bitcast(mybir.dt.uint8)

            oma_ts, o_ts, m_ts, a_ts = [], [], [], []
            for t in range(NT):
                oma_t = iop.tile([128, OMA_W], mybir.dt.uint8, tag=f"oma{t}")
                oma_ts.append(oma_t)
                o_ts.append(oma_t[:, 0:2000].bitcast(BF16))
                m_ts.append(oma_t[:, 2000:4000].bitcast(BF16))
                a_ts.append(oma_t[:, 4000:6000].bitcast(BF16))

            # progressive feature pieces so the first gram starts early
            P0, P1, P2 = 512, 1536, 4608
            dma(out=cpack_t[:], in_=cpack_d[:])

            def dma_hp(lo, hi):
                for j in (0, 1):
                    dma(out=hp8u[:, j:j + 1, lo:hi],
                        in_=hp_d[:, j * EXT + lo:j * EXT + hi])

            def dma_oma(t, lo, hi):
                # split across queues so one tile isn't serialized on a ring
                rsl = slice(t * 128, (t + 1) * 128)
                step = (hi - lo + 1) // 2
                for x in range(lo, hi, step):
                    e = min(x + step, hi)
                    dma(out=oma_ts[t][:, x:e], in_=oma_d[rsl, x:e])

            dma_hp(0, P0)
            dma_oma(0, 0, 4000)     # o+m of tile 0: first ACT/DVE cekl work
            dma_hp(P0, P1)
            dma_oma(0, 4000, 6016)
            dma_oma(1, 0, 4000)
            dma_hp(P1, P2)
            dma_oma(1, 4000, 6016)
            dma_oma(2, 0, 4000)
            dma_hp(P2, EXT)
            dma_oma(2, 4000, 6016)
            dma_oma(3, 0, 4000)
            dma_oma(3, 4000, 6016)

            out_sb = vecp.tile([128, OUT_W], F32, tag="out_sb")
            rs_sl = out_sb[:, 0:NSLOT]
            rs_x = out_sb[:, 28:29]          # second slot of the split pair 0
            pos_sb = out_sb[:, 29:33]
            st_sb = out_sb[:, 33:OUT_W]
            cs_sb = vecp.tile([1, EXT], F32, tag="cs_sb")
            pscr = vecp.tile([128, 128], F32, tag="pscr")
            dummy_a = vecp.tile([128, 1000], BF16, tag="dummy_a")  # ACT-only
            dummy_v = vecp.tile([128, 1000], BF16, tag="dummy_v")  # DVE-only

            et_tiles = {}
            em_ts = {}

            def emit_gram(idx):
                l, c, a, b = PAIRS[idx]
                w = b - a
                s_l = 128 * l
                gp = gpp.tile([128, CHW], F32, tag="gp")
                et_tiles[idx] = (gp, None)
                lhsT = hp8[:, :, s_l:s_l + 128]
                for sub in range(0, w, 512):
                    n = min(512, w - sub)
                    d0 = a + sub <= s_l < a + sub + n
                    nc.tensor.matmul(gp[:, sub:sub + n], lhsT,
                                     hp8[:, :, a + sub:a + sub + n],
                                     perf_mode=DR, start=True, stop=not d0,
                                     skip_group_check=True)
                    if d0:
                        off = s_l - a
                        nc.tensor.matmul(gp[:, off:off + 128], negidb_t[:],
                                         identb_t[:], start=False, stop=True,
                                         skip_group_check=True)
                return gp

            def emit_pos(idx, gp):
                l, c, a, b = PAIRS[idx]
                p0 = 128 * l + 4096
                if l < 4 and a <= p0 < b:
                    off = p0 - a
                    nc.vector.scalar_tensor_tensor(
                        out=pscr[:], in0=gp[:, off:off + 128], scalar=1.0,
                        in1=ident_t[:], op0=ALU.mult, op1=ALU.mult,
                        accum_out=pos_sb[:, l:l + 1])

            def emit_exp_act(idx, gp, split=False):
                l, c, a, b = PAIRS[idx]
                w = b - a
                e_t = etp.tile([128, CHW], BF16, tag="et")
                et_tiles[idx] = (gp, e_t)
                if split:
                    nc.scalar.activation(e_t[:, 0:512], gp[:, 0:512], AF.Exp,
                                         scale=ACT_SCALE, accum_out=rs_x[:])
                    nc.scalar.activation(e_t[:, 512:w], gp[:, 512:w], AF.Exp,
                                         scale=ACT_SCALE,
                                         accum_out=rs_sl[:, idx:idx + 1])
                else:
                    nc.scalar.activation(e_t[:, :w], gp[:, :w], AF.Exp,
                                         scale=ACT_SCALE,
                                         accum_out=rs_sl[:, idx:idx + 1])

            def emit_cekl_unit(t, kind):
                src = (o_ts[t], o_ts[t], m_ts[t], a_ts[t])[kind]
                slot = st_sb[:, kind * 4 + t:kind * 4 + t + 1]
                on_act = (t, kind) in CEKL_ON_ACT
                if kind == 2:
                    em_t = emp.tile([128, 1000], BF16, tag="em")
                    em_ts[t] = em_t
                    dst = em_t[:]
                else:
                    dst = (dummy_a if on_act else dummy_v)[:, 0:1000]
                if on_act:
                    scale = 1.0 if kind in (0, 3) else 0.25
                    nc.scalar.activation(dst, src[:], AF.Exp, scale=scale,
                                         accum_out=slot)
                else:
                    s = LOG2E / 32.0 if kind in (0, 3) else LOG2E / 128.0
                    b1, b2, b3, _ = _poly_coefs(s)
                    b4col = 1 if kind in (0, 3) else 2
                    q_t = qtp.tile([128, 1000], F32, tag="q")
                    nc.vector._custom_dve(
                        EXP2PA, out=q_t[:], in0=src[:],
                        in1=b4c_t[:, b4col:b4col + 1], s0=b1, s1=b2, imm2=b3)
                    nc.vector._custom_dve(
                        EXP2PB, out=dst, in0=q_t[:], s0=PC0_32,
                        accum_out=slot)

            def emit_cekl_pp(t):
                d_t = scrp.tile([128, 1000], BF16, tag="d")
                nc.vector.tensor_sub(d_t[:], m_ts[t][:], o_ts[t][:])
                nc.vector.scalar_tensor_tensor(
                    out=dummy_v[:, 0:1000], in0=d_t[:], scalar=1.0,
                    in1=em_ts[t][:], op0=ALU.mult, op1=ALU.mult,
                    accum_out=st_sb[:, 16 + t:17 + t])

            def emit_colsums(c):
                for r in range(3 * c, min(3 * c + 3, NREG)):
                    if r not in REGION_PIECES:
                        continue
                    pieces = REGION_PIECES[r]
                    ct = csp.tile([128, RGW], F32, tag="cs")
                    full0 = (pieces[0][1] == RGW * r
                             and pieces[0][2] == RGW * (r + 1))
                    if not full0:
                        nc.tensor.matmul(ct[:], onesb_t[:], zerob_t[:],
                                         start=True, stop=False,
                                         skip_group_check=True)
                    for k, (idx, p0_, p1_) in enumerate(pieces):
                        _, _, a, _ = PAIRS[idx]
                        e_t = et_tiles[idx][1]
                        nc.tensor.matmul(
                            ct[:, p0_ - RGW * r:p1_ - RGW * r],
                            onesb_t[:], e_t[:, p0_ - a:p1_ - a],
                            start=(k == 0 and full0),
                            stop=(k == len(pieces) - 1),
                            skip_group_check=True)
                    nc.vector.tensor_copy(
                        cs_sb[0:1, RGW * r:RGW * (r + 1)], ct[0:1, :])
                if c % 2 == 1 or c == NCHK - 1:
                    lo = RGW * 3 * (c - 1 if c % 2 == 1 else c)
                    hi = min(RGW * 3 * (c + 1), EXT)
                    dma(out=cs_d[0:1, lo:hi], in_=cs_sb[0:1, lo:hi])

            # unit emission order: DVE units early (DMA-dependent only) in
            # tile order matching DMA arrival; ACT units fill gram gaps.
            dve_units = [(0, 1), (0, 3), (1, 1), (1, 0), (1, 3), (2, 1),
                         (2, 0), (2, 3), (3, 1), (3, 0), (3, 3)]
            act_units = [(1, 2), (2, 2), (3, 2)]
            pair_of_chunk = [[i for i, p in enumerate(PAIRS) if p[1] == c]
                             for c in range(NCHK)]
            pp_done = 0
            for c in range(NCHK):
                for k, idx in enumerate(pair_of_chunk[c]):
                    gp = emit_gram(idx)
                    if k == 0 and c >= 1:
                        emit_colsums(c - 1)
                    emit_pos(idx, gp)
                    if idx == 0:
                        # tile-0 cekl first: ACT starts on DMA, not on PE
                        emit_cekl_unit(0, 0)
                        emit_cekl_unit(0, 2)
                    emit_exp_act(idx, gp, split=(idx == 0))
                    if idx == 0:
                        for _ in range(2):
                            if dve_units:
                                emit_cekl_unit(*dve_units.pop(0))
                    elif k == len(pair_of_chunk[c]) // 2 and c >= 1:
                        if act_units:
                            emit_cekl_unit(*act_units.pop(0))
                        if dve_units:
                            emit_cekl_unit(*dve_units.pop(0))
                if c >= 1:
                    if act_units:
                        emit_cekl_unit(*act_units.pop(0))
                    if dve_units:
                        emit_cekl_unit(*dve_units.pop(0))
                    if pp_done < NT and pp_done in em_ts:
                        emit_cekl_pp(pp_done)
                        pp_done += 1
            emit_colsums(NCHK - 1)
            for u in act_units:
                emit_cekl_unit(*u)
            for u in dve_units:
                emit_cekl_unit(*u)
            while pp_done < NT:
                if pp_done in em_ts:
                    emit_cekl_pp(pp_done)
                pp_done += 1

            dma(out=out_d[:], in_=out_sb[:])

    nc.compile()
    return nc


_NC = None


def _get_nc():
    global _NC
    if _NC is None:
        _NC = _build_module()
    return _NC


_HOST = {}


def _prep_inputs(output, target, master_net_pred, feat_pooled,
                 feat_pooled_masked, output_adv, target_adv):
    o = np.asarray(output, dtype=np.float32)
    m = np.asarray(master_net_pred, dtype=np.float32)
    a = np.asarray(output_adv, dtype=np.float32)
    tg = np.asarray(target).astype(np.int64)
    ta = np.asarray(target_adv).astype(np.int64)
    f0 = np.asarray(feat_pooled, dtype=np.float32)
    f1 = np.asarray(feat_pooled_masked, dtype=np.float32)
    feats = np.concatenate([f0, f1], axis=0)  # [2B, D]
    feats = feats / np.linalg.norm(feats, axis=1, keepdims=True)
    feats = feats * np.float32(np.sqrt(S2))

    _HOST["GO"] = np.take_along_axis(o, tg[:, None], axis=1)[:, 0]
    _HOST["GA"] = np.take_along_axis(a, ta[:, None], axis=1)[:, 0]

    o_bf = o.astype(ml_dtypes.bfloat16)
    m_bf = m.astype(ml_dtypes.bfloat16)
    a_bf = a.astype(ml_dtypes.bfloat16)

    in_maps = []
    for cc in range(NCORES):
        sl = slice(cc * RB, (cc + 1) * RB)
        rolled = np.roll(feats, -RB * cc, axis=0)
        ext = np.concatenate([rolled, rolled[:EXT - N2]], axis=0)  # [8704, D]
        f8 = np.ascontiguousarray(ext.T).astype(ml_dtypes.float8_e4m3)
        hp = np.concatenate([f8[0:128], f8[128:256]], axis=1)  # [128, 2*EXT]
        oma = np.zeros((RB, OMA_W), dtype=np.uint8)
        oma[:, 0:2000] = np.ascontiguousarray(o_bf[sl]).view(np.uint8)
        oma[:, 2000:4000] = np.ascontiguousarray(m_bf[sl]).view(np.uint8)
        oma[:, 4000:6000] = np.ascontiguousarray(a_bf[sl]).view(np.uint8)
        in_maps.append({"oma": oma, "hp": hp.view(np.uint8)})
    return in_maps


def _combine(results):
    S = np.zeros(N2, dtype=np.float64)
    pos_full = np.zeros(N2, dtype=np.float64)
    arp = np.arange(128)
    for cc, rr in enumerate(results):
        rs = rr["out"][:, 0:NSLOT].astype(np.float64)
        rs[:, 0] += rr["out"][:, 28].astype(np.float64)
        cs = rr["cs"].reshape(-1).astype(np.float64)   # [EXT]
        pos = rr["out"][:, 29:33].astype(np.float64)
        for idx, (l, c, a, b) in enumerate(PAIRS):
            rows = (RB * cc + 128 * l + arp) % N2
            np.add.at(S, rows, rs[:, idx])
        gcols = (np.arange(EXT) + RB * cc) % N2
        np.add.at(S, gcols, cs)
        for l in range(4):
            i = RB * cc + 128 * l + arp
            pos_full[i] = pos[:, l]
            pos_full[i + B] = pos[:, l]
    pos_logit = pos_full * ACT_SCALE
    nce_mean = float(np.mean(np.log(S) - pos_logit))

    # CE / KL / focal / adv from per-row stats
    sts = [r["out"][:, 33:OUT_W] for r in results]
    S1 = np.concatenate([st[:, 0:4].T.reshape(-1) for st in sts])
    ST = np.concatenate([st[:, 4:8].T.reshape(-1) for st in sts])
    SM = np.concatenate([st[:, 8:12].T.reshape(-1) for st in sts])
    SA = np.concatenate([st[:, 12:16].T.reshape(-1) for st in sts])
    PP = np.concatenate([st[:, 16:20].T.reshape(-1) for st in sts])
    S1, ST, SM, SA, PP = (x.astype(np.float64)
                          for x in (S1, ST, SM, SA, PP))
    GO = _HOST["GO"].astype(np.float64)
    GA = _HOST["GA"].astype(np.float64)
    ce = np.log(S1) - GO
    adv = np.log(SA) - GA
    kl = PP / (KL_TEMP * SM) - np.log(SM) + np.log(ST)
    pt = np.exp(-ce)
    gamma = np.where(pt < 0.2, 5.0, np.where(pt < 0.5, 3.0, 1.0))
    foc = ((1.0 - pt) ** gamma) * ce
    loss = (KL_INTERP * KL_TEMP * KL_TEMP) * np.mean(kl) / C \
        + (1.0 - KL_INTERP) * np.mean(ce) + nce_mean \
        + np.mean(foc) + np.mean(adv)
    return np.asarray([loss], dtype=np.float32)


def kernel(**inputs):
    in_maps = _prep_inputs(**inputs)
    out = run_bass_kernel_spmd(_get_nc(), in_maps,
                               core_ids=list(range(NCORES)))
    return _combine(out.results)


if __name__ == "__main__":
    rng = np.random.default_rng(0)
    ins = {
        "output": rng.standard_normal((B, C), dtype=np.float32),
        "target": rng.integers(0, C, size=(B,)),
        "master_net_pred": rng.standard_normal((B, C), dtype=np.float32),
        "feat_pooled": rng.standard_normal((B, D), dtype=np.float32),
        "feat_pooled_masked": rng.standard_normal((B, D), dtype=np.float32),
        "output_adv": rng.standard_normal((B, C), dtype=np.float32),
        "target_adv": rng.integers(0, C, size=(B,)),
    }
    print(kernel(**ins))
